# revision 1
# baseline (speedup 1.0000x reference)
"""Trainium2 Bass kernel for ModalitySpecificLocalSelfAttention (7x7 local window).

Strategy (8 NeuronCores, spatial-parallel over H):
  - Each core owns a 16-row stripe of the 128x128 image; k/v paths get a
    3-row halo (22 rows total). 1x1 convs run as PE matmuls with BN scale
    folded into the weights on the host (float32r, full PE rate).
  - Local attention per 8x16 pixel block: one matmul Q_blockT @ K_neigh
    gives a dense [128, 308] score matrix covering the 14x22 padded
    neighborhood; a constant additive mask (-1e30 off-window) + exp
    reproduces the reference's zero-pad softmax semantics exactly.
  - Value aggregation: PE-transpose A and V_neigh to put the neighborhood
    dim on partitions (bf16), then one accumulating matmul -> [C, 128].
  - Final conv: W_a @ attn + W_x @ x accumulated in PSUM + bias.
"""

import sys

for _p in ("/opt/trn_rl_repo", "/root/.axon_site/_ro/trn_rl_repo"):
    if _p not in sys.path:
        sys.path.append(_p)

import ml_dtypes
import numpy as np

import concourse.bass as bass
from concourse import mybir
from concourse.bass_utils import run_bass_kernel_spmd

F32 = mybir.dt.float32
F32R = mybir.dt.float32r
BF16 = mybir.dt.bfloat16

C = 128
H = 128
W = 128
NCORES = 8
RPC = H // NCORES          # 16 rows per core
PAD = 3
HALO = RPC + 2 * PAD       # 22 rows incl halo
WP = W + 2 * PAD           # 134 padded width
BR, BC = 8, 16             # pixel block 8 rows x 16 cols = 128 pixels
NBR, NBC = RPC // BR, W // BC
NR, NC_ = BR + 2 * PAD, BC + 2 * PAD  # neighborhood 14 x 22
NN = NR * NC_              # 308
NPIX = RPC * W             # 2048 pixels per core
NHALO = HALO * W           # 2816

EXP_SHIFT = -16.0          # constant bias inside exp (sim range ~[0, 32])
CH = 512                   # conv matmul N-chunk (one PSUM bank of f32)


NN2 = 384                  # attention width padded to XBAR/transpose granularity


def _build_program():
    """Raw-Bass SPMD program (one NeuronCore's 16-row stripe).

    Single-wait discipline: this walrus build rejects instructions with
    more than one semaphore wait, so every cross-engine dependency is a
    standalone ``wait_ge`` and each tracked instruction increments its
    engine's semaphore.  The schedule is planned in Python, then emitted
    into the per-engine streams of an ``nc.Block``.

    Block phase runs in 4 groups of 4 blocks (one 8-row block-row half)
    so softmax ops amortize instruction overhead; A/V transposes run on
    the DMA XBAR (bf16), not the PE.
    """
    nc = bass.Bass("TRN2", target_bir_lowering=False, debug=False)

    # ---- DRAM I/O ----
    xs_d = nc.dram_tensor("xs", [C, NHALO], BF16, kind="ExternalInput").ap()
    W_NAMES = ("wq1t", "wq2t", "wk1t", "wk2t", "wvt", "wat", "wxt")
    B_NAMES = ("bq1", "bq2", "bk1", "bk2", "bv", "bo")
    wall_d = nc.dram_tensor("wall", [C, 8 * C], BF16, kind="ExternalInput").ap()
    ball_d = nc.dram_tensor("ball", [C, 8], F32, kind="ExternalInput").ap()
    smask_d = nc.dram_tensor("smask", [C, NBR, NN2], BF16,
                             kind="ExternalInput").ap()
    y_d = nc.dram_tensor("y", [C, NPIX], F32, kind="ExternalOutput").ap()

    # ---- SBUF ----
    sb = lambda name, shape, dt: nc.alloc_sbuf_tensor(name, list(shape), dt).ap()
    xs = sb("xs_sb", [C, NHALO], BF16)
    k1 = sb("k1_sb", [C, NHALO], BF16)
    q1 = sb("q1_sb", [C, NPIX], BF16)
    q = sb("q_sb", [C, RPC, W], BF16)
    kpad = sb("kpad_sb", [C, HALO, WP], BF16)
    vpad = sb("vpad_sb", [C, HALO, WP], BF16)
    attn = sb("attn_sb", [C, RPC, W], BF16)
    wall = sb("wall_sb", [C, 8 * C], BF16)
    ball = sb("ball_sb", [C, 8], F32)
    w_sb = {n: wall[:, bass.ts(k, C)] for k, n in enumerate(W_NAMES)}
    b_sb = {n: ball[:, k:k + 1] for k, n in enumerate(B_NAMES)}
    id16 = wall[:, bass.ts(7, C)]
    oobc = ball[:, 6:8]
    smask = sb("smask_sb", [C, NBR, NN2], BF16)
    eshift = sb("eshift_sb", [C, 1], F32)
    qb4 = sb("qb4_sb", [C, 2, 4, BR * BC], BF16)
    kn4 = sb("kn4_sb", [C, 2, 4, NN2], BF16)
    vn4 = sb("vn4_sb", [C, 2, 4, NN2], BF16)
    am4 = sb("am4_sb", [C, 2, 4, NN2], BF16)
    e4 = sb("e4_sb", [C, 2, 4, NN2], BF16)
    a16 = sb("a16_sb", [C, 2, 4, NN2], BF16)
    z4 = sb("z4_sb", [C, 2, 4], F32)
    zs4 = sb("zs4_sb", [C, 2, 4], F32)
    rz4 = sb("rz4_sb", [C, 2, 4], F32)
    at16 = sb("at16_sb", [C, 2, 4, 3, C], BF16)
    vt16 = sb("vt16_sb", [C, 2, 4, 3, C], BF16)
    yt = sb("yt_sb", [C, 2, CH], F32)

    # ---- PSUM: two 4-bank tensors = all 8 banks ----
    # conv phase: 8 rotating [C, 512] chunk slots
    # block group g: parity tensor holds 4x S ([.., :384]) + 4x AV ([.., 384:])
    # o-conv: alternating [C, 512] slots
    PS = [nc.alloc_psum_tensor(f"ps{i}", [C, 4, CH], F32).ap()
          for i in range(2)]

    # ---- semaphores ----
    sem_names = tuple(
        ["sde", "sdw", "sdt", "sdout0", "sdout1", "sp", "sa", "sv", "sg"]
        + [f"sdx{j}" for j in range(6)]
        + [f"sdta{p}{i}" for p in range(2) for i in range(4)]
        + [f"sdtv{p}{i}" for p in range(2) for i in range(4)])
    sems = {n: nc.alloc_semaphore(n) for n in sem_names}

    ENGS = ("sync", "pe", "act", "dve", "gp")
    plan = {e: [] for e in ENGS}
    cnt = {n: 0 for n in sem_names}

    def op(eng, fn, sem, inc=1):
        plan[eng].append(("op", fn, sem, inc))
        if sem:
            cnt[sem] += inc
            return cnt[sem]
        return None

    def wait(eng, sem, val):
        if val and val > 0:
            plan[eng].append(("w", sem, val))

    RELU = mybir.ActivationFunctionType.Relu
    IDENT = mybir.ActivationFunctionType.Identity
    EXP = mybir.ActivationFunctionType.Exp

    # ---- input DMAs: early group feeds k1/q1 start, rest follows ----
    def dma_in(sem, dst, srcd):
        return op("sync",
                  lambda d=dst, s=srcd: nc.sync.dma_start(out=d, in_=s),
                  sem, 16)

    dma_in("sde", wall, wall_d)
    SDE_ALL = dma_in("sde", ball, ball_d)
    for i in range((NHALO + CH - 1) // CH):
        n = min(CH, NHALO - i * CH)
        dma_in(f"sdx{i}", xs[:, bass.ds(i * CH, n)],
               xs_d[:, bass.ds(i * CH, n)])
    SDW_ALL = dma_in("sdw", smask, smask_d)

    # ---- init memsets (zero borders of kpad/vpad; zero pad cols of kn/vn) ----
    for t in (kpad, vpad):
        op("gp", lambda tf=t.rearrange("p r w -> p (r w)"):
            nc.gpsimd.memset(tf, 0.0), "sg")
    for t in (kn4, vn4):
        op("gp", lambda tf=t.rearrange("p a b c -> p (a b c)"):
            nc.gpsimd.memset(tf, 0.0), "sg")
    MEMSETS = cnt["sg"]
    op("dve", lambda: nc.vector.memset(eshift, EXP_SHIFT), "sv")

    # ---- convs: emitted chunk-by-chunk in a custom order ----
    conv_list = [
        ("k1", "wk1t", xs, 0, NHALO, k1, "bk1"),
        ("q1", "wq1t", xs, PAD * W, NPIX, q1, "bq1"),
        ("v", "wvt", xs, 0, NHALO, vpad, "bv"),
        ("k2", "wk2t", k1, 0, NHALO, kpad, "bk2"),
        ("q2", "wq2t", q1, 0, NPIX, q, "bq2"),
    ]
    mm_done, epi_done = {}, {}
    last_slot_epi = {}           # (tensor_idx, slot) -> epi mark
    last_tensor_epi = [None, None]   # tensor_idx -> (sem, value)

    def emit_conv_chunk(ci, j, tidx, slot):
        cname, wn, rhs, roff, ntot, dst, bn = conv_list[ci]
        n = min(CH, ntot - j * CH)
        ps = PS[tidx][:, slot, :]
        src_conv = {"k2": 0, "q2": 1}.get(cname)
        if cname in ("k1", "v"):
            wait("pe", f"sdx{j}", 16)
        elif cname == "q1":
            wait("pe", f"sdx{j}", 16)
            wait("pe", f"sdx{j + 1}", 16)
        if src_conv is not None:
            m_ = epi_done.get((src_conv, j))
            if m_:
                wait("pe", m_[0], m_[1])
        m_ = last_slot_epi.get((tidx, slot))
        if m_:
            wait("pe", m_[0], m_[1])
        mm_done[(ci, j)] = op(
            "pe",
            lambda p=ps[:, :n], w_=w_sb[wn],
                   r=rhs[:, bass.ds(roff + j * CH, n)]:
                nc.tensor.matmul(p, w_, r, start=True, stop=True),
            "sp")
        wait("act", "sp", mm_done[(ci, j)])
        if cname in ("v", "k2"):
            nr = n // W
            r0 = (j * CH) // W
            if cname == "v":
                wait("act", "sg", MEMSETS)
            val = ("sa", op(
                "act",
                lambda o=dst[:, r0:r0 + nr, PAD:PAD + W],
                       p=ps[:, :n].rearrange("p (r w) -> p r w", w=W),
                       b=b_sb[bn]:
                    nc.scalar.activation(o, p, RELU, bias=b),
                "sa"))
        elif cname == "q2":
            nr = n // W
            r0 = (j * CH) // W
            val = ("sa", op(
                "act",
                lambda o=dst[:, r0:r0 + nr, :],
                       p=ps[:, :n].rearrange("p (r w) -> p r w", w=W),
                       b=b_sb[bn]:
                    nc.scalar.activation(o, p, RELU, bias=b),
                "sa"))
        else:
            val = ("sa", op(
                "act",
                lambda o=dst[:, bass.ds(j * CH, n)], p=ps[:, :n],
                       b=b_sb[bn]:
                    nc.scalar.activation(o, p, RELU, bias=b),
                "sa"))
        epi_done[(ci, j)] = val
        last_slot_epi[(tidx, slot)] = val
        last_tensor_epi[tidx] = val

    wait("pe", "sde", SDE_ALL)
    wait("act", "sde", SDE_ALL)

    # group-0/1 prereqs first; tensor A gets the first 16 chunks so the
    # block phase (which starts on A) frees it early, B takes the tail 10
    conv_order = (
        [(0, j) for j in range(4)] + [(1, j) for j in range(2)]
        + [(2, j) for j in range(4)] + [(3, j) for j in range(4)]
        + [(4, j) for j in range(2)] + [(0, j) for j in range(4, 6)]
        + [(1, j) for j in range(2, 4)] + [(2, j) for j in range(4, 6)]
        + [(3, j) for j in range(4, 6)] + [(4, j) for j in range(2, 4)]
    )
    for idx, (ci, j) in enumerate(conv_order):
        if idx < 16:
            emit_conv_chunk(ci, j, 0, idx % 4)
        else:
            emit_conv_chunk(ci, j, 1, (idx - 16) % 4)

    # block-phase DVE prereqs (smask DMA, kpad/vpad border memsets)
    wait("dve", "sdw", SDW_ALL)
    wait("dve", "sg", MEMSETS)

    # ---- attention blocks: 4 groups of 4, software-pipelined ----
    sdone, expdone, muldone = {}, {}, {}
    qbdone, vndone, kndone, avdone, acdone = {}, {}, {}, {}, {}
    vtm, atm = {}, {}

    def grp_geom(grp):
        return grp // 2, grp % 2, grp % 2   # br, half, parity

    def st_gathers(grp):
        br, half, par = grp_geom(grp)
        r0 = br * BR
        # prereqs: q2 chunks for this block-row; v/k2 chunks for rows used
        wait("gp", *epi_done[(4, 2 * br + 1)])
        if grp >= 2:
            wait("gp", "sp", sdone[grp - 2])
        for i in range(4):
            c0 = (half * 4 + i) * BC
            qbdone[grp] = op(
                "gp",
                lambda d=qb4[:, par, i, :].rearrange("p (r w) -> p r w", r=BR),
                       s_=q[:, r0:r0 + BR, c0:c0 + BC]:
                    nc.gpsimd.tensor_copy(d, s_),
                "sg")
        wait("gp", *epi_done[(2, 3 if br == 0 else 5)])
        for i in range(4):
            c0 = (half * 4 + i) * BC
            if grp >= 2:
                wait("gp", f"sdtv{par}{i}", vtm[(grp - 2, i)])
            vndone[grp] = op(
                "gp",
                lambda d=vn4[:, par, i, 0:NN].rearrange(
                           "p (r w) -> p r w", r=NR),
                       s_=vpad[:, r0:r0 + NR, c0:c0 + NC_]:
                    nc.gpsimd.tensor_copy(d, s_),
                "sg")
        wait("dve", *epi_done[(3, 3 if br == 0 else 5)])
        if grp >= 2:
            wait("dve", "sp", sdone[grp - 2])
        for i in range(4):
            c0 = (half * 4 + i) * BC
            kndone[grp] = op(
                "dve",
                lambda d=kn4[:, par, i, 0:NN].rearrange(
                           "p (r w) -> p r w", r=NR),
                       s_=kpad[:, r0:r0 + NR, c0:c0 + NC_]:
                    nc.vector.tensor_copy(d, s_),
                "sv")

    def st_s(grp):
        br, half, par = grp_geom(grp)
        psX = PS[par]
        if last_tensor_epi[par]:
            wait("pe", *last_tensor_epi[par])
        wait("pe", "sg", qbdone[grp])
        wait("pe", "sv", kndone[grp])
        if grp >= 2:
            # S region previously read by exp (ACT); and the av region of
            # the SAME banks read by attn-copy — same-bank PE-write with a
            # concurrent ACT-read crashes the PSUM bank (P10), so wait for
            # the whole-bank readers, not just the S-region ones
            wait("pe", "sa", acdone[grp - 2])
        for i in range(4):
            sdone[(grp, i)] = sdone[grp] = op(
                "pe",
                lambda o=psX[:, i, 0:NN2], l=qb4[:, par, i, :],
                       r=kn4[:, par, i, :]:
                    nc.tensor.matmul(o, l, r, start=True, stop=True),
                "sp")

    def st_vtrans(grp):
        br, half, par = grp_geom(grp)
        wait("sync", "sg", vndone[grp])
        if grp >= 2:
            wait("sync", "sp", avdone[grp - 2])
        for i in range(4):
            vtm[(grp, i)] = op(
                "sync",
                lambda o=vt16[:, par, i], s_=vn4[:, par, i, :]:
                    nc.sync.dma_start(out=o, in_=s_, transpose=True),
                f"sdtv{par}{i}", 16)
        for i in range(4):
            wait("sync", f"sdtv{par}{i}", vtm[(grp, i)])

    def st_softmax(grp):
        br, half, par = grp_geom(grp)
        psX = PS[par]
        # ACT: exp straight off PSUM, per block
        if grp >= 2:
            wait("act", "sv", muldone[grp - 2])
        for i in range(4):
            wait("act", "sp", sdone[(grp, i)])
            expdone[(grp, i)] = expdone[grp] = op(
                "act",
                lambda o=e4[:, par, i, :], i_=psX[:, i, 0:NN2]:
                    nc.scalar.activation(o, i_, EXP, bias=eshift),
                "sa")
        # DVE: per-block chain am -> zsum -> +oob -> recip -> normalize,
        # emitted as a wavefront so same-engine RAW waits never stall
        amm, zrm, zam, rcm = {}, {}, {}, {}

        def dve_stage(s, i):
            if s == 0:
                wait("dve", "sa", expdone[(grp, i)])
                amm[i] = op(
                    "dve",
                    lambda o=am4[:, par, i, :], i0=e4[:, par, i, :],
                           i1=smask[:, br, :]:
                        nc.vector.tensor_mul(o, i0, i1),
                    "sv")
            elif s == 1:
                wait("dve", "sv", amm[i])
                zrm[i] = op(
                    "dve",
                    lambda o=z4[:, par, i:i + 1], i_=am4[:, par, i, :]:
                        nc.vector.reduce_sum(o, i_,
                                             axis=mybir.AxisListType.X),
                    "sv")
            elif s == 2:
                wait("dve", "sv", zrm[i])
                zam[i] = op(
                    "dve",
                    lambda o=zs4[:, par, i:i + 1], i_=z4[:, par, i:i + 1],
                           s_=oobc[:, br:br + 1]:
                        nc.vector.tensor_scalar_add(o, i_, s_),
                    "sv")
            elif s == 3:
                wait("dve", "sv", zam[i])
                rcm[i] = op(
                    "dve",
                    lambda o=rz4[:, par, i:i + 1], i_=zs4[:, par, i:i + 1]:
                        nc.vector.reciprocal(o, i_),
                    "sv")
            else:
                wait("dve", "sv", rcm[i])
                muldone[(grp, i)] = muldone[grp] = op(
                    "dve",
                    lambda o=a16[:, par, i, :], i_=am4[:, par, i, :],
                           s_=rz4[:, par, i:i + 1]:
                        nc.vector.tensor_scalar_mul(o, i_, s_),
                    "sv")

        for wv in range(9):
            for i in range(4):
                s = wv - i
                if 0 <= s < 5:
                    dve_stage(s, i)

    def st_atrans(grp):
        br, half, par = grp_geom(grp)
        # gate enqueue order: same sems are reused by group grp+2, and the
        # next enqueue must come after this group's AV consumed at16
        if grp >= 2:
            wait("sync", "sp", avdone[grp - 2])
        for i in range(4):
            wait("sync", "sv", muldone[(grp, i)])
            atm[(grp, i)] = op(
                "sync",
                lambda o=at16[:, par, i], s_=a16[:, par, i, :]:
                    nc.sync.dma_start(out=o, in_=s_, transpose=True),
                f"sdta{par}{i}", 16)
        for i in range(4):
            wait("sync", f"sdta{par}{i}", atm[(grp, i)])

    def st_av(grp):
        br, half, par = grp_geom(grp)
        psX = PS[par]
        if grp >= 2:
            wait("pe", "sa", acdone[grp - 2])
        for i in range(4):
            wait("pe", f"sdtv{par}{i}", vtm[(grp, i)])
            wait("pe", f"sdta{par}{i}", atm[(grp, i)])
            for ch in range(3):
                avdone[grp] = op(
                    "pe",
                    lambda o=psX[:, i, NN2:CH], l=vt16[:, par, i, ch, :],
                           r=at16[:, par, i, ch, :],
                           st=(ch == 0), sp_=(ch == 2):
                        nc.tensor.matmul(o, l, r, start=st, stop=sp_),
                    "sp")

    def st_accopy(grp):
        br, half, par = grp_geom(grp)
        psX = PS[par]
        r0 = br * BR
        wait("act", "sp", avdone[grp])
        acdone[grp] = op(
            "act",
            lambda o=attn[:, r0:r0 + BR,
                          half * 64:half * 64 + 64].rearrange(
                              "p r (b w) -> p b r w", w=BC),
                   i_=psX[:, :, NN2:CH].rearrange(
                       "p b (r w) -> p b r w", w=BC):
                nc.scalar.copy(o, i_),
            "sa")

    for grp in range(4):
        st_gathers(grp)
        if grp >= 2:
            st_accopy(grp - 2)
        st_s(grp)
        st_vtrans(grp)
        if grp >= 1:
            st_softmax(grp - 1)
            st_atrans(grp - 1)
            st_av(grp - 1)
    st_softmax(3)
    st_atrans(3)
    st_av(3)
    st_accopy(2)
    st_accopy(3)

    # ---- output conv ----
    attn_flat = attn.rearrange("p r w -> p (r w)")
    oc_done, yt_done = {}, {}
    wait("pe", "sa", acdone[3])
    for i in range(NPIX // CH):
        pq = i % 2
        ps = PS[pq][:, 0, :]
        if i >= 2:
            wait("pe", "sa", yt_done[i - 2])
        op("pe",
           lambda o=ps, l=w_sb["wat"], r=attn_flat[:, bass.ts(i, CH)]:
               nc.tensor.matmul(o, l, r, start=True, stop=False),
           "sp")
        oc_done[i] = op(
            "pe",
            lambda o=ps, l=w_sb["wxt"],
                   r=xs[:, bass.ds(PAD * W + i * CH, CH)]:
                nc.tensor.matmul(o, l, r, start=False, stop=True),
            "sp")
        wait("act", "sp", oc_done[i])
        if i >= 2:
            wait("act", f"sdout{pq}", 16 * (i // 2))
        yt_done[i] = op(
            "act",
            lambda o=yt[:, pq, :], i_=ps, b=b_sb["bo"]:
                nc.scalar.activation(o, i_, IDENT, bias=b),
            "sa")
        wait("sync", "sa", yt_done[i])
        op("sync",
           lambda o=y_d[:, bass.ts(i, CH)], i_=yt[:, pq, :]:
               nc.sync.dma_start(out=o, in_=i_),
           f"sdout{pq}", 16)

    # ---- tail: wait everything before the final barrier ----
    wait("sync", "sp", cnt["sp"])
    wait("sync", "sa", cnt["sa"])
    wait("sync", "sv", cnt["sv"])
    wait("sync", "sg", cnt["sg"])
    wait("sync", "sdout0", cnt["sdout0"])
    wait("sync", "sdout1", cnt["sdout1"])
    wait("sync", "sde", SDE_ALL)
    wait("sync", "sdw", SDW_ALL)
    for j in range(6):
        wait("sync", f"sdx{j}", cnt[f"sdx{j}"])
    for p_ in range(2):
        for i_ in range(4):
            wait("sync", f"sdta{p_}{i_}", cnt[f"sdta{p_}{i_}"])
            wait("sync", f"sdtv{p_}{i_}", cnt[f"sdtv{p_}{i_}"])

    # ---- emit ----
    def run(eng_name, eng_obj):
        hwm = {}
        for item in plan[eng_name]:
            if item[0] == "w":
                _, s_, v = item
                if hwm.get(s_, 0) >= v:
                    continue
                hwm[s_] = v
                eng_obj.wait_ge(sems[s_], v)
            else:
                _, fn, s_, inc = item
                inst = fn()
                if s_:
                    inst.then_inc(sems[s_], inc)

    with nc.Block() as block:
        @block.sync
        def _(e):
            run("sync", e)

        @block.tensor
        def _(e):
            run("pe", e)

        @block.scalar
        def _(e):
            run("act", e)

        @block.vector
        def _(e):
            run("dve", e)

        @block.gpsimd
        def _(e):
            run("gp", e)

    with nc.Block() as block2:
        @block2.sync
        def _(e):
            for n in sem_names:
                nc.sync.sem_clear(sems[n])

    return nc


_PROGRAM = None


def _host_inputs(x, w_q1, s_q1, b_q1, w_q2, s_q2, b_q2,
                 w_k1, s_k1, b_k1, w_k2, s_k2, b_k2,
                 w_v, s_v, b_v, w_o, s_o, b_o):
    """Per-core input dicts (numpy) for the SPMD program."""
    def foldT(w, s):
        return np.ascontiguousarray((s[:, None] * w).T.astype(ml_dtypes.bfloat16))

    wq1t, wq2t = foldT(w_q1, s_q1), foldT(w_q2, s_q2)
    wk1t, wk2t = foldT(w_k1, s_k1), foldT(w_k2, s_k2)
    wvt = foldT(w_v, s_v)
    wo = s_o[:, None] * w_o
    wat = np.ascontiguousarray(wo[:, :C].T.astype(ml_dtypes.bfloat16))
    wxt = np.ascontiguousarray(wo[:, C:].T.astype(ml_dtypes.bfloat16))

    col = lambda b: np.ascontiguousarray(b.astype(np.float32)[:, None])

    # window-validity over the 14x22 neighborhood, per block pixel
    valid = np.zeros((BR * BC, NR, NC_), bool)
    for r in range(BR):
        for c in range(BC):
            p = r * BC + c
            valid[p, r:r + 7, c:c + 7] = True

    X = np.asarray(x, np.float32).reshape(C, H, W)
    wall = np.concatenate(
        [wq1t, wq2t, wk1t, wk2t, wvt, wat, wxt,
         np.eye(C, dtype=ml_dtypes.bfloat16)], axis=1)
    shared = dict(wall=np.ascontiguousarray(wall))

    e16v = np.float32(np.exp(EXP_SHIFT))
    in_maps = []
    for core in range(NCORES):
        h0 = core * RPC
        xsb = np.zeros((C, HALO, W), np.float32)
        lo, hi = h0 - PAD, h0 + RPC + PAD
        slo, shi = max(lo, 0), min(hi, H)
        xsb[:, slo - lo:shi - lo] = X[:, slo:shi]

        # per-block-row multiplicative 0/1 mask (0 for off-window, OOB-row,
        # and pad cols) and out-of-image-row Z compensation
        smask = np.zeros((NBR, BR * BC, NN2), np.float32)
        oobc = np.zeros((NBR, BR * BC), np.float32)
        for brr in range(NBR):
            rowok = np.array([0 <= h0 + brr * BR + ri - PAD < H
                              for ri in range(NR)])
            m = (valid & rowok[None, :, None]).astype(np.float32)
            smask[brr, :, :NN] = m.reshape(BR * BC, NN)
            for r in range(BR):
                n_oob = sum(1 for i in range(7)
                            if not (0 <= h0 + brr * BR + r - PAD + i < H))
                oobc[brr, r * BC:(r + 1) * BC] = 7 * n_oob * e16v
        m = dict(shared)
        m["xs"] = np.ascontiguousarray(
            xsb.reshape(C, NHALO).astype(ml_dtypes.bfloat16))
        m["smask"] = np.ascontiguousarray(
            smask.transpose(1, 0, 2).astype(ml_dtypes.bfloat16))
        m["ball"] = np.ascontiguousarray(np.concatenate(
            [col(b_q1), col(b_q2), col(b_k1), col(b_k2), col(b_v),
             col(b_o), oobc.T.astype(np.float32)], axis=1))
        in_maps.append(m)
    return in_maps


def kernel(**inputs):
    global _PROGRAM
    if _PROGRAM is None:
        _PROGRAM = _build_program()
    in_maps = _host_inputs(**{k: np.asarray(v) for k, v in inputs.items()})
    res = run_bass_kernel_spmd(_PROGRAM, in_maps, core_ids=list(range(NCORES)))
    stripes = [np.asarray(r["y"]).reshape(C, RPC, W) for r in res.results]
    return np.concatenate(stripes, axis=1).reshape(1, C, H, W)


if __name__ == "__main__":
    rng = np.random.default_rng(0)
    fake = {"x": rng.standard_normal((1, C, H, W), np.float32)}
    for n in ("q1", "q2", "k1", "k2", "v", "o"):
        cin = 2 * C if n == "o" else C
        fake["w_" + n] = rng.standard_normal((C, cin), np.float32) / np.sqrt(cin)
        fake["s_" + n] = rng.uniform(0.5, 1.5, C).astype(np.float32)
        fake["b_" + n] = (rng.standard_normal(C) * 0.1).astype(np.float32)
    out = kernel(**fake)
    print("kernel output", out.shape, out.dtype)

